# revision 28
# baseline (speedup 1.0000x reference)
"""GCN message-passing kernel for Trainium2 (Bass/Tile), 8-core SPMD.

Problem: nn_GCN_1 — 3-layer per-bond-type graph conv:
    H0 = embed[N]                                  # [B, n, d]
    Es = E + I; d = rowsum(Es)^-1/2; En = D Es D   # per (b, t)
    H_{l+1} = relu(En @ H_l @ W_l[t])              # l = 0..2
    out = H3                                       # [B, T, n, d]

Sharding: data-parallel over batch B=32 across 8 cores (4 batches/core);
weights / embedding replicated.

Per-core device algorithm (per b, t), exploiting En = D.Es.D with
D = diag(d) so no free-dim broadcasts are ever needed:
    Z_0 = D H0                       (per-partition scale)
    for l in 0..2:
        G'T = Z_l^T Es^T             (matmul: lhsT=Z_l [j,d], rhs=Es^T [j,i])
        O   = G' W_l                 (matmul: lhsT=G'T [d,i], rhs=W [d,e])
        Z_{l+1} = relu(D^2 O)        (= D * relu(D * (Es Z W)); D > 0)
    out = relu(D O_2)  on the last layer
Es^T is materialized once per (b, t) via 16 PE transposes and reused by
all 3 layers. The embedding gather is a one-hot matmul on the PE.
"""

import os
import sys

if "/opt/trn_rl_repo" not in sys.path:
    sys.path.insert(0, "/opt/trn_rl_repo")

import numpy as np

import concourse.bacc as bacc
import concourse.bass as bass
import concourse.mybir as mybir
import concourse.tile as tile
from concourse.bass_utils import run_bass_kernel_spmd

NCORES = 8
B, T, NN, D, V = 32, 3, 512, 128, 21
BC = B // NCORES  # batches per core
NT = NN // 128    # node tiles of 128

F32 = mybir.dt.float32
BF16 = mybir.dt.bfloat16

# Matmul dtype for the O(n^2 d) work: "bf16" (fast path) or "f32" (fallback)
MM_MODE = os.environ.get("KERNEL_MM_MODE", "bf16")

_module_cache = {}


def _build_module(mm_mode: str) -> bass.Bass:
    nc = bacc.Bacc(
        "TRN2",
        target_bir_lowering=False,
        debug=False,
        enable_asserts=False,
        num_devices=NCORES,
    )
    e = nc.dram_tensor("e", [BC, T, NN, NN], F32, kind="ExternalInput")
    oh = nc.dram_tensor("oh", [BC, V, NN], F32, kind="ExternalInput")
    emb = nc.dram_tensor("emb", [V, D], F32, kind="ExternalInput")
    w = nc.dram_tensor("w", [3, T, D, D], F32, kind="ExternalInput")
    ident = nc.dram_tensor("ident", [128, 128], F32, kind="ExternalInput")
    out = nc.dram_tensor("out", [BC, T, NN, D], F32, kind="ExternalOutput")

    # DRAM views: node dim split into (tile ii, partition p)
    e_v = e.rearrange("b t (ii p) j -> b t p ii j", p=128)
    out_v = out.rearrange("b t (ii p) e -> b t p ii e", p=128)
    w_v = w.rearrange("l t d e -> d l t e")

    mm_dt = BF16 if mm_mode == "bf16" else F32

    def mm_view(ap):
        # view an f32 AP as float32r for the fast fp32 matmul path
        if mm_mode == "f32r":
            return ap.bitcast(mybir.dt.float32r)
        return ap

    with tile.TileContext(nc) as tc:
        with (
            tc.tile_pool(name="const", bufs=1) as cpool,
            tc.tile_pool(name="ohp", bufs=2) as ohpool,
            tc.tile_pool(name="h0p", bufs=2) as h0pool,
            tc.tile_pool(name="ep", bufs=2) as epool,
            tc.tile_pool(name="estp", bufs=2) as estpool,
            tc.tile_pool(name="degp", bufs=2) as degpool,
            tc.tile_pool(name="zp", bufs=3) as zpool,
            tc.tile_pool(name="gtp", bufs=2) as gtpool,
            tc.tile_pool(name="ptp", bufs=4, space="PSUM") as ptpool,
            tc.tile_pool(name="pgp", bufs=2, space="PSUM") as pgpool,
            tc.tile_pool(name="pop", bufs=2, space="PSUM") as popool,
        ):
            emb_sb = cpool.tile([V, D], F32, name="emb_sb")
            nc.sync.dma_start(emb_sb[:], emb.ap())
            ident_sb = cpool.tile([128, 128], F32, name="ident_sb")
            nc.sync.dma_start(ident_sb[:], ident.ap())
            w_sb = cpool.tile([128, 9 * D], F32, name="w_sb")
            nc.sync.dma_start(
                w_sb[:].rearrange("p (l t e) -> p l t e", l=3, t=3), w_v
            )
            if mm_mode == "bf16":
                w_mm = cpool.tile([128, 9 * D], BF16, name="w_mm")
                nc.vector.tensor_copy(w_mm[:], w_sb[:])
            else:
                w_mm = w_sb

            for b in range(BC):
                # H0 = onehot(N[b]) @ embed, via PE
                oh_sb = ohpool.tile([V, NN], F32, name="oh_sb")
                nc.sync.dma_start(oh_sb[:], oh.ap()[b])
                ph = popool.tile([128, NT * D], F32, name="ph", tag="po")
                for ii in range(NT):
                    nc.tensor.matmul(
                        ph[:, ii * D : (ii + 1) * D],
                        lhsT=oh_sb[:, ii * 128 : (ii + 1) * 128],
                        rhs=emb_sb[:],
                        start=True,
                        stop=True,
                    )
                h0 = h0pool.tile([128, NT * D], F32, name="h0")
                nc.scalar.copy(h0[:], ph[:])

                for t in range(T):
                    # ---- load E[b, t] (1 MB, contiguous) ----
                    e_sb = epool.tile([128, NT * NN], F32, name="e_sb", tag="e")
                    nc.sync.dma_start(
                        e_sb[:].rearrange("p (ii j) -> p ii j", ii=NT), e_v[b, t]
                    )
                    # ---- degrees: d2 = 1/(rowsum+1), dd = sqrt(d2) ----
                    s = degpool.tile([128, NT], F32, name="s", tag="s")
                    nc.vector.tensor_reduce(
                        s[:],
                        e_sb[:].rearrange("p (ii j) -> p ii j", ii=NT),
                        axis=mybir.AxisListType.X,
                        op=mybir.AluOpType.add,
                    )
                    s1 = degpool.tile([128, NT], F32, name="s1", tag="s1")
                    nc.vector.tensor_scalar_add(s1[:], s[:], 1.0)
                    d2 = degpool.tile([128, NT], F32, name="d2", tag="d2")
                    nc.vector.reciprocal(d2[:], s1[:])
                    dd = degpool.tile([128, NT], F32, name="dd", tag="dd")
                    nc.scalar.sqrt(dd[:], d2[:])

                    # ---- Es^T via PE transposes; +I on diagonal blocks ----
                    est = estpool.tile([128, NT * NN], mm_dt, name="est", tag="est")
                    for jj in range(NT):
                        pt = ptpool.tile([128, 512], F32, name="pt", tag="pt")
                        for ii in range(NT):
                            nc.tensor.transpose(
                                pt[:, ii * 128 : (ii + 1) * 128],
                                e_sb[:, ii * NN + jj * 128 : ii * NN + jj * 128 + 128],
                                ident_sb[:],
                            )
                        base = jj * NN
                        dlo, dhi = jj * 128, jj * 128 + 128
                        if dlo > 0:
                            nc.vector.tensor_copy(
                                est[:, base : base + dlo], pt[:, :dlo]
                            )
                        nc.vector.tensor_add(
                            est[:, base + dlo : base + dhi],
                            pt[:, dlo:dhi],
                            ident_sb[:],
                        )
                        if dhi < NN:
                            nc.vector.tensor_copy(
                                est[:, base + dhi : base + NN], pt[:, dhi:NN]
                            )

                    # ---- 3 chained gconv layers ----
                    z = None
                    for l in range(3):
                        if l == 0:
                            z = zpool.tile([128, NT * D], mm_dt, name="z", tag="z")
                            for ii in range(NT):
                                nc.vector.tensor_scalar_mul(
                                    z[:, ii * D : (ii + 1) * D],
                                    h0[:, ii * D : (ii + 1) * D],
                                    dd[:, ii : ii + 1],
                                )
                        # G'T[d, i] = sum_j Z[j, d] * EsT[j, i]
                        pgt = pgpool.tile([128, NN], F32, name="pgt", tag="pg")
                        for jj in range(NT):
                            nc.tensor.matmul(
                                pgt[:],
                                lhsT=mm_view(z[:, jj * D : (jj + 1) * D]),
                                rhs=mm_view(est[:, jj * NN : (jj + 1) * NN]),
                                start=(jj == 0),
                                stop=(jj == NT - 1),
                            )
                        gt = gtpool.tile([128, NN], mm_dt, name="gt", tag="gt")
                        nc.scalar.copy(gt[:], pgt[:])
                        # O[i, e] = sum_d G'T[d, i] * W[d, e]
                        po = popool.tile([128, NT * D], F32, name="po", tag="po")
                        wsl = w_mm[:, (l * T + t) * D : (l * T + t + 1) * D]
                        for ii in range(NT):
                            nc.tensor.matmul(
                                po[:, ii * D : (ii + 1) * D],
                                lhsT=mm_view(gt[:, ii * 128 : (ii + 1) * 128]),
                                rhs=mm_view(wsl),
                                start=True,
                                stop=True,
                            )
                        # Z_{l+1} = relu(D^2 O); last layer: out = relu(D O)
                        last = l == 2
                        znext = zpool.tile(
                            [128, NT * D], F32 if last else mm_dt, name="zn", tag="z"
                        )
                        sc = dd if last else d2
                        for ii in range(NT):
                            nc.scalar.activation(
                                znext[:, ii * D : (ii + 1) * D],
                                po[:, ii * D : (ii + 1) * D],
                                mybir.ActivationFunctionType.Relu,
                                scale=sc[:, ii : ii + 1],
                            )
                        z = znext

                    nc.scalar.dma_start(
                        out_v[b, t], z[:].rearrange("p (ii e) -> p ii e", ii=NT)
                    )

    nc.compile()
    return nc


def _build_module_v2() -> bass.Bass:
    """bf16 pipeline, E shipped as bf16 from the host (halves HBM traffic).

    Z-chaining: Z_0 = D H0, Z_{l+1} = relu(D^2 (Es Z_l W_l)) for l<2, and the
    final output is relu(D (Es Z_2 W_2)) — all diagonal scalings live in the
    relu epilogue (per-partition broadcast over i) or the initial H0 scale
    (per-partition over j), so Es^T itself is copied PSUM->SBUF unscaled at
    plain-copy rate, split between DVE and ACT. Diagonal +I adds and the H0
    scale run on the otherwise idle gpsimd."""
    nc = bacc.Bacc(
        "TRN2",
        target_bir_lowering=False,
        debug=False,
        enable_asserts=False,
        num_devices=NCORES,
    )
    e = nc.dram_tensor("e", [BC, T, NN, NN], BF16, kind="ExternalInput")
    oh = nc.dram_tensor("oh", [BC, V, NN], BF16, kind="ExternalInput")
    emb = nc.dram_tensor("emb", [V, D], BF16, kind="ExternalInput")
    w = nc.dram_tensor("w", [3, T, D, D], BF16, kind="ExternalInput")
    ident = nc.dram_tensor("ident", [128, 128], BF16, kind="ExternalInput")
    out = nc.dram_tensor("out", [BC, T, NN, D], F32, kind="ExternalOutput")

    e_v = e.rearrange("b t (ii p) j -> b t p ii j", p=128)
    out_v = out.rearrange("b t (ii p) e -> b t p ii e", p=128)
    w_v = w.rearrange("l t d e -> d l t e")

    with tile.TileContext(nc) as tc:
        with (
            tc.tile_pool(name="const", bufs=1) as cpool,
            tc.tile_pool(name="ohp", bufs=2) as ohpool,
            tc.tile_pool(name="h0p", bufs=2) as h0pool,
            tc.tile_pool(name="ep", bufs=3) as epool,
            tc.tile_pool(name="estp", bufs=3) as estpool,
            tc.tile_pool(name="degp", bufs=3) as degpool,
            tc.tile_pool(name="zp", bufs=6) as zpool,
            tc.tile_pool(name="gtp", bufs=3) as gtpool,
            tc.tile_pool(name="ptp", bufs=1, space="PSUM") as ptpool,
            tc.tile_pool(name="pgp", bufs=3, space="PSUM") as pgpool,
            tc.tile_pool(name="pop", bufs=3, space="PSUM") as popool,
        ):
            emb_sb = cpool.tile([V, D], BF16, name="emb_sb")
            nc.sync.dma_start(emb_sb[:], emb.ap())
            identb = cpool.tile([128, 128], BF16, name="identb")
            nc.sync.dma_start(identb[:], ident.ap())
            w_bf = cpool.tile([128, 9 * D], BF16, name="w_bf")
            nc.sync.dma_start(
                w_bf[:].rearrange("p (l t e) -> p l t e", l=3, t=3), w_v
            )

            h0_by_b = {}

            def emit_prologue(st):
                """DMA + diagonal +I for (b, t); H0 block at each new b."""
                b, t = st["b"], st["t"]
                if t == 0:
                    oh_sb = ohpool.tile([V, NN], BF16, name="oh_sb")
                    nc.sync.dma_start(oh_sb[:], oh.ap()[b])
                    ph = popool.tile([128, NT * D], F32, name="ph", tag="po")
                    for ii in range(NT):
                        nc.tensor.matmul(
                            ph[:, ii * D : (ii + 1) * D],
                            lhsT=oh_sb[:, ii * 128 : (ii + 1) * 128],
                            rhs=emb_sb[:],
                            start=True,
                            stop=True,
                        )
                    h0 = h0pool.tile([128, NT * D], BF16, name="h0")
                    nc.scalar.copy(h0[:], ph[:])
                    h0_by_b[b] = h0
                e_bf = epool.tile([128, NT * NN], BF16, name="e_bf", tag="e")
                nc.sync.dma_start(
                    e_bf[:].rearrange("p (ii j) -> p ii j", ii=NT), e_v[b, t]
                )
                st["e_bf"] = e_bf

            def emit_reduce_chunk(st, ii):
                """Partial rowsum of E+I for node-block ii (small op, easy
                for the scheduler to slot between the relu/copy traffic)."""
                if ii == 0:
                    st["s1"] = degpool.tile([128, NT], F32, name="s1", tag="s1")
                nc.vector.tensor_reduce(
                    st["s1"][:, ii : ii + 1],
                    st["e_bf"][:].rearrange("p (ii j) -> p ii j", ii=NT)[
                        :, ii : ii + 1, :
                    ],
                    axis=mybir.AxisListType.X,
                    op=mybir.AluOpType.add,
                )

            def emit_degrees(st):
                """d2 = 1/rowsum, dd = sqrt(d2)."""
                d2 = degpool.tile([128, NT], F32, name="d2", tag="d2")
                nc.vector.reciprocal(d2[:], st["s1"][:])
                dd = degpool.tile([128, NT], F32, name="dd", tag="dd")
                nc.scalar.sqrt(dd[:], d2[:])
                st["d2"], st["dd"] = d2, dd

            def emit_diag(st):
                """+ I on diagonal blocks of e_bf (idle gpsimd), so Es^T
                includes the self-loops; the rowsum adds its +1 separately."""
                for ii in range(NT):
                    blk = slice(ii * NN + ii * 128, ii * NN + ii * 128 + 128)
                    nc.gpsimd.tensor_add(
                        st["e_bf"][:, blk], st["e_bf"][:, blk], identb[:]
                    )

            def emit_tgroup(st, jj):
                """4 PE transposes for column-block jj + its Es^T copy."""
                if jj == 0:
                    st["pt"] = ptpool.tile([128, NT * NN], BF16, name="pt", tag="pt")
                    st["est"] = estpool.tile(
                        [128, NT * NN], BF16, name="est", tag="est"
                    )
                pt, est, e_bf = st["pt"], st["est"], st["e_bf"]
                for ii in range(NT):
                    off = jj * NN + ii * 128
                    nc.tensor.transpose(
                        pt[:, off : off + 128],
                        e_bf[:, ii * NN + jj * 128 : ii * NN + jj * 128 + 128],
                        identb[:],
                    )
                sl = slice(jj * NN, (jj + 1) * NN)
                if jj % 2 == 0:
                    nc.vector.tensor_copy(est[:, sl], pt[:, sl])
                else:
                    nc.scalar.copy(est[:, sl], pt[:, sl])

            def emit_z0(st):
                """Z_0 = dd * H0 (one fused DVE broadcast multiply)."""
                z0 = zpool.tile([128, NT * D], BF16, name="z0", tag="z")
                nc.vector.tensor_tensor(
                    z0[:].rearrange("p (ii e) -> p ii e", ii=NT),
                    h0_by_b[st["b"]][:].rearrange("p (ii e) -> p ii e", ii=NT),
                    st["dd"][:].to_broadcast([128, NT, D]),
                    op=mybir.AluOpType.mult,
                )
                st["h"] = z0

            def emit_big(st, l):
                """G'T = (Es Z_l)^T: 4 accumulating matmuls into one bank."""
                pgt = pgpool.tile([128, NN], F32, name="pgt", tag="pg")
                h, est = st["h"], st["est"]
                for jj in range(NT):
                    nc.tensor.matmul(
                        pgt[:],
                        lhsT=h[:, jj * D : (jj + 1) * D],
                        rhs=est[:, jj * NN : (jj + 1) * NN],
                        start=(jj == 0),
                        stop=(jj == NT - 1),
                    )
                st["pgt"] = pgt

            def emit_gt(st, l):
                gt = gtpool.tile([128, NN], BF16, name="gt", tag="gt")
                nc.scalar.copy(gt[:], st["pgt"][:])
                st["gt"] = gt

            def emit_wmm(st, l):
                po = popool.tile([128, NT * D], F32, name="po", tag="po")
                gt = st["gt"]
                wsl = w_bf[:, (l * T + st["t"]) * D : (l * T + st["t"] + 1) * D]
                for ii in range(NT):
                    nc.tensor.matmul(
                        po[:, ii * D : (ii + 1) * D],
                        lhsT=gt[:, ii * 128 : (ii + 1) * 128],
                        rhs=wsl,
                        start=True,
                        stop=True,
                    )
                st["po"] = po

            def emit_relu(st, l):
                """Z_{l+1} = relu(po)*d2 (dd on the last layer), then out."""
                last = l == 2
                hn = zpool.tile(
                    [128, NT * D], F32 if last else BF16, name="hn", tag="z"
                )
                sc = st["dd"] if last else st["d2"]
                nc.vector.scalar_tensor_tensor(
                    hn[:].rearrange("p (ii e) -> p ii e", ii=NT),
                    st["po"][:].rearrange("p (ii e) -> p ii e", ii=NT),
                    0.0,
                    sc[:].to_broadcast([128, NT, D]),
                    op0=mybir.AluOpType.max,
                    op1=mybir.AluOpType.mult,
                )
                st["h"] = hn
                if last:
                    nc.scalar.dma_start(
                        out_v[st["b"], st["t"]],
                        hn[:].rearrange("p (ii e) -> p ii e", ii=NT),
                    )

            # 2-deep software pipeline: iteration k runs stage S0 (loads /
            # Es^T DMA-transpose / degrees / Z0) for bt_k, stage S1 (layer 0
            # + layer-1 matmul) for bt_{k-1}, and stage S2 (layer 1 epilogue
            # + layer 2 + store) for bt_{k-2}. Two independent layer chains
            # keep the PE busy across every cross-engine handoff, which also
            # keeps the HAM clock-gate warm.
            bts = [(b, t) for b in range(BC) for t in range(T)]
            sts = [{"b": b, "t": t} for b, t in bts]
            for k in range(len(bts) + 2):
                st = sts[k] if k < len(bts) else None
                A = sts[k - 1] if 1 <= k <= len(bts) else None
                B = sts[k - 2] if 2 <= k <= len(bts) + 1 else None
                if st:
                    emit_prologue(st)
                    emit_diag(st)
                if B:
                    emit_gt(B, 1)
                if st:
                    emit_tgroup(st, 0)
                    emit_reduce_chunk(st, 0)
                if A:
                    emit_big(A, 0)
                if B:
                    emit_wmm(B, 1)
                if st:
                    emit_tgroup(st, 1)
                    emit_reduce_chunk(st, 1)
                if A:
                    emit_gt(A, 0)
                if B:
                    emit_relu(B, 1)
                    emit_big(B, 2)
                if st:
                    emit_tgroup(st, 2)
                    emit_reduce_chunk(st, 2)
                if A:
                    emit_wmm(A, 0)
                if B:
                    emit_gt(B, 2)
                if A:
                    emit_relu(A, 0)
                if st:
                    emit_tgroup(st, 3)
                    emit_reduce_chunk(st, 3)
                if B:
                    emit_wmm(B, 2)
                if A:
                    emit_big(A, 1)
                if B:
                    emit_relu(B, 2)
                if st:
                    emit_degrees(st)
                    emit_z0(st)

    nc.compile()
    return nc


def _get_module(mm_mode: str) -> bass.Bass:
    if mm_mode not in _module_cache:
        if mm_mode == "bf16":
            _module_cache[mm_mode] = _build_module_v2()
        else:
            _module_cache[mm_mode] = _build_module(mm_mode)
    return _module_cache[mm_mode]


last_results = None


def kernel(**inputs) -> np.ndarray:
    N = np.asarray(inputs["N"])
    E = np.asarray(inputs["E"], dtype=np.float32)
    embed = np.ascontiguousarray(np.asarray(inputs["embed"], dtype=np.float32))
    W = np.ascontiguousarray(
        np.stack(
            [
                np.asarray(inputs["W1"], dtype=np.float32),
                np.asarray(inputs["W2"], dtype=np.float32),
                np.asarray(inputs["W3"], dtype=np.float32),
            ]
        )
    )  # [3, T, D, D]
    oh = (N[:, None, :] == np.arange(V)[None, :, None]).astype(np.float32)
    ident = np.eye(128, dtype=np.float32)

    if MM_MODE == "bf16":
        import ml_dtypes

        bf = ml_dtypes.bfloat16
        E = E.astype(bf)
        embed = embed.astype(bf)
        W = W.astype(bf)
        oh = oh.astype(bf)
        ident = ident.astype(bf)

    nc = _get_module(MM_MODE)
    in_maps = []
    for c in range(NCORES):
        sl = slice(c * BC, (c + 1) * BC)
        in_maps.append(
            {
                "e": np.ascontiguousarray(E[sl]),
                "oh": np.ascontiguousarray(oh[sl]),
                "emb": embed,
                "w": W,
                "ident": ident,
            }
        )

    trace = os.environ.get("KERNEL_TRACE", "") == "1"
    res = run_bass_kernel_spmd(
        nc,
        in_maps,
        core_ids=list(range(NCORES)),
        trace=trace,
    )
    global last_results
    last_results = res
    return np.concatenate([r["out"] for r in res.results], axis=0)


# revision 30
# speedup vs baseline: 1.0505x; 1.0505x over previous
"""GCN message-passing kernel for Trainium2 (Bass/Tile), 8-core SPMD.

Problem: nn_GCN_1 — 3-layer per-bond-type graph conv:
    H0 = embed[N]                                  # [B, n, d]
    Es = E + I; d = rowsum(Es)^-1/2; En = D Es D   # per (b, t)
    H_{l+1} = relu(En @ H_l @ W_l[t])              # l = 0..2
    out = H3                                       # [B, T, n, d]

Sharding: data-parallel over batch B=32 across 8 cores (4 batches/core);
weights / embedding replicated.

Per-core device algorithm (per b, t), exploiting En = D.Es.D with
D = diag(d) so no free-dim broadcasts are ever needed:
    Z_0 = D H0                       (per-partition scale)
    for l in 0..2:
        G'T = Z_l^T Es^T             (matmul: lhsT=Z_l [j,d], rhs=Es^T [j,i])
        O   = G' W_l                 (matmul: lhsT=G'T [d,i], rhs=W [d,e])
        Z_{l+1} = relu(D^2 O)        (= D * relu(D * (Es Z W)); D > 0)
    out = relu(D O_2)  on the last layer
Es^T is materialized once per (b, t) via 16 PE transposes and reused by
all 3 layers. The embedding gather is a one-hot matmul on the PE.
"""

import os
import sys

if "/opt/trn_rl_repo" not in sys.path:
    sys.path.insert(0, "/opt/trn_rl_repo")

import numpy as np

import concourse.bacc as bacc
import concourse.bass as bass
import concourse.mybir as mybir
import concourse.tile as tile
from concourse.bass_utils import run_bass_kernel_spmd

NCORES = 8
B, T, NN, D, V = 32, 3, 512, 128, 21
BC = B // NCORES  # batches per core
NT = NN // 128    # node tiles of 128

F32 = mybir.dt.float32
BF16 = mybir.dt.bfloat16

# Matmul dtype for the O(n^2 d) work: "bf16" (fast path) or "f32" (fallback)
MM_MODE = os.environ.get("KERNEL_MM_MODE", "bf16")

_module_cache = {}


def _build_module(mm_mode: str) -> bass.Bass:
    nc = bacc.Bacc(
        "TRN2",
        target_bir_lowering=False,
        debug=False,
        enable_asserts=False,
        num_devices=NCORES,
    )
    e = nc.dram_tensor("e", [BC, T, NN, NN], F32, kind="ExternalInput")
    oh = nc.dram_tensor("oh", [BC, V, NN], F32, kind="ExternalInput")
    emb = nc.dram_tensor("emb", [V, D], F32, kind="ExternalInput")
    w = nc.dram_tensor("w", [3, T, D, D], F32, kind="ExternalInput")
    ident = nc.dram_tensor("ident", [128, 128], F32, kind="ExternalInput")
    out = nc.dram_tensor("out", [BC, T, NN, D], F32, kind="ExternalOutput")

    # DRAM views: node dim split into (tile ii, partition p)
    e_v = e.rearrange("b t (ii p) j -> b t p ii j", p=128)
    out_v = out.rearrange("b t (ii p) e -> b t p ii e", p=128)
    w_v = w.rearrange("l t d e -> d l t e")

    mm_dt = BF16 if mm_mode == "bf16" else F32

    def mm_view(ap):
        # view an f32 AP as float32r for the fast fp32 matmul path
        if mm_mode == "f32r":
            return ap.bitcast(mybir.dt.float32r)
        return ap

    with tile.TileContext(nc) as tc:
        with (
            tc.tile_pool(name="const", bufs=1) as cpool,
            tc.tile_pool(name="ohp", bufs=2) as ohpool,
            tc.tile_pool(name="h0p", bufs=2) as h0pool,
            tc.tile_pool(name="ep", bufs=2) as epool,
            tc.tile_pool(name="estp", bufs=2) as estpool,
            tc.tile_pool(name="degp", bufs=2) as degpool,
            tc.tile_pool(name="zp", bufs=3) as zpool,
            tc.tile_pool(name="gtp", bufs=2) as gtpool,
            tc.tile_pool(name="ptp", bufs=4, space="PSUM") as ptpool,
            tc.tile_pool(name="pgp", bufs=2, space="PSUM") as pgpool,
            tc.tile_pool(name="pop", bufs=2, space="PSUM") as popool,
        ):
            emb_sb = cpool.tile([V, D], F32, name="emb_sb")
            nc.sync.dma_start(emb_sb[:], emb.ap())
            ident_sb = cpool.tile([128, 128], F32, name="ident_sb")
            nc.sync.dma_start(ident_sb[:], ident.ap())
            w_sb = cpool.tile([128, 9 * D], F32, name="w_sb")
            nc.sync.dma_start(
                w_sb[:].rearrange("p (l t e) -> p l t e", l=3, t=3), w_v
            )
            if mm_mode == "bf16":
                w_mm = cpool.tile([128, 9 * D], BF16, name="w_mm")
                nc.vector.tensor_copy(w_mm[:], w_sb[:])
            else:
                w_mm = w_sb

            for b in range(BC):
                # H0 = onehot(N[b]) @ embed, via PE
                oh_sb = ohpool.tile([V, NN], F32, name="oh_sb")
                nc.sync.dma_start(oh_sb[:], oh.ap()[b])
                ph = popool.tile([128, NT * D], F32, name="ph", tag="po")
                for ii in range(NT):
                    nc.tensor.matmul(
                        ph[:, ii * D : (ii + 1) * D],
                        lhsT=oh_sb[:, ii * 128 : (ii + 1) * 128],
                        rhs=emb_sb[:],
                        start=True,
                        stop=True,
                    )
                h0 = h0pool.tile([128, NT * D], F32, name="h0")
                nc.scalar.copy(h0[:], ph[:])

                for t in range(T):
                    # ---- load E[b, t] (1 MB, contiguous) ----
                    e_sb = epool.tile([128, NT * NN], F32, name="e_sb", tag="e")
                    nc.sync.dma_start(
                        e_sb[:].rearrange("p (ii j) -> p ii j", ii=NT), e_v[b, t]
                    )
                    # ---- degrees: d2 = 1/(rowsum+1), dd = sqrt(d2) ----
                    s = degpool.tile([128, NT], F32, name="s", tag="s")
                    nc.vector.tensor_reduce(
                        s[:],
                        e_sb[:].rearrange("p (ii j) -> p ii j", ii=NT),
                        axis=mybir.AxisListType.X,
                        op=mybir.AluOpType.add,
                    )
                    s1 = degpool.tile([128, NT], F32, name="s1", tag="s1")
                    nc.vector.tensor_scalar_add(s1[:], s[:], 1.0)
                    d2 = degpool.tile([128, NT], F32, name="d2", tag="d2")
                    nc.vector.reciprocal(d2[:], s1[:])
                    dd = degpool.tile([128, NT], F32, name="dd", tag="dd")
                    nc.scalar.sqrt(dd[:], d2[:])

                    # ---- Es^T via PE transposes; +I on diagonal blocks ----
                    est = estpool.tile([128, NT * NN], mm_dt, name="est", tag="est")
                    for jj in range(NT):
                        pt = ptpool.tile([128, 512], F32, name="pt", tag="pt")
                        for ii in range(NT):
                            nc.tensor.transpose(
                                pt[:, ii * 128 : (ii + 1) * 128],
                                e_sb[:, ii * NN + jj * 128 : ii * NN + jj * 128 + 128],
                                ident_sb[:],
                            )
                        base = jj * NN
                        dlo, dhi = jj * 128, jj * 128 + 128
                        if dlo > 0:
                            nc.vector.tensor_copy(
                                est[:, base : base + dlo], pt[:, :dlo]
                            )
                        nc.vector.tensor_add(
                            est[:, base + dlo : base + dhi],
                            pt[:, dlo:dhi],
                            ident_sb[:],
                        )
                        if dhi < NN:
                            nc.vector.tensor_copy(
                                est[:, base + dhi : base + NN], pt[:, dhi:NN]
                            )

                    # ---- 3 chained gconv layers ----
                    z = None
                    for l in range(3):
                        if l == 0:
                            z = zpool.tile([128, NT * D], mm_dt, name="z", tag="z")
                            for ii in range(NT):
                                nc.vector.tensor_scalar_mul(
                                    z[:, ii * D : (ii + 1) * D],
                                    h0[:, ii * D : (ii + 1) * D],
                                    dd[:, ii : ii + 1],
                                )
                        # G'T[d, i] = sum_j Z[j, d] * EsT[j, i]
                        pgt = pgpool.tile([128, NN], F32, name="pgt", tag="pg")
                        for jj in range(NT):
                            nc.tensor.matmul(
                                pgt[:],
                                lhsT=mm_view(z[:, jj * D : (jj + 1) * D]),
                                rhs=mm_view(est[:, jj * NN : (jj + 1) * NN]),
                                start=(jj == 0),
                                stop=(jj == NT - 1),
                            )
                        gt = gtpool.tile([128, NN], mm_dt, name="gt", tag="gt")
                        nc.scalar.copy(gt[:], pgt[:])
                        # O[i, e] = sum_d G'T[d, i] * W[d, e]
                        po = popool.tile([128, NT * D], F32, name="po", tag="po")
                        wsl = w_mm[:, (l * T + t) * D : (l * T + t + 1) * D]
                        for ii in range(NT):
                            nc.tensor.matmul(
                                po[:, ii * D : (ii + 1) * D],
                                lhsT=mm_view(gt[:, ii * 128 : (ii + 1) * 128]),
                                rhs=mm_view(wsl),
                                start=True,
                                stop=True,
                            )
                        # Z_{l+1} = relu(D^2 O); last layer: out = relu(D O)
                        last = l == 2
                        znext = zpool.tile(
                            [128, NT * D], F32 if last else mm_dt, name="zn", tag="z"
                        )
                        sc = dd if last else d2
                        for ii in range(NT):
                            nc.scalar.activation(
                                znext[:, ii * D : (ii + 1) * D],
                                po[:, ii * D : (ii + 1) * D],
                                mybir.ActivationFunctionType.Relu,
                                scale=sc[:, ii : ii + 1],
                            )
                        z = znext

                    nc.scalar.dma_start(
                        out_v[b, t], z[:].rearrange("p (ii e) -> p ii e", ii=NT)
                    )

    nc.compile()
    return nc


def _build_module_v2() -> bass.Bass:
    """bf16 pipeline, E shipped as bf16 from the host (halves HBM traffic).

    Z-chaining: Z_0 = D H0, Z_{l+1} = relu(D^2 (Es Z_l W_l)) for l<2, and the
    final output is relu(D (Es Z_2 W_2)) — all diagonal scalings live in the
    relu epilogue (per-partition broadcast over i) or the initial H0 scale
    (per-partition over j), so Es^T itself is copied PSUM->SBUF unscaled at
    plain-copy rate, split between DVE and ACT. Diagonal +I adds and the H0
    scale run on the otherwise idle gpsimd."""
    nc = bacc.Bacc(
        "TRN2",
        target_bir_lowering=False,
        debug=False,
        enable_asserts=False,
        num_devices=NCORES,
    )
    e = nc.dram_tensor("e", [BC, T, NN, NN], BF16, kind="ExternalInput")
    oh = nc.dram_tensor("oh", [BC, V, NN], BF16, kind="ExternalInput")
    emb = nc.dram_tensor("emb", [V, D], BF16, kind="ExternalInput")
    w = nc.dram_tensor("w", [3, T, D, D], BF16, kind="ExternalInput")
    ident = nc.dram_tensor("ident", [128, 128], BF16, kind="ExternalInput")
    out = nc.dram_tensor("out", [BC, T, NN, D], F32, kind="ExternalOutput")

    e_v = e.rearrange("b t (ii p) j -> b t p ii j", p=128)
    out_v = out.rearrange("b t (ii p) e -> b t p ii e", p=128)
    w_v = w.rearrange("l t d e -> d l t e")

    with tile.TileContext(nc) as tc:
        with (
            tc.tile_pool(name="const", bufs=1) as cpool,
            tc.tile_pool(name="ohp", bufs=2) as ohpool,
            tc.tile_pool(name="h0p", bufs=2) as h0pool,
            tc.tile_pool(name="ep", bufs=4) as epool,
            tc.tile_pool(name="estp", bufs=4) as estpool,
            tc.tile_pool(name="degp", bufs=4) as degpool,
            tc.tile_pool(name="zp", bufs=6) as zpool,
            tc.tile_pool(name="gtp", bufs=3) as gtpool,
            tc.tile_pool(name="ptp", bufs=1, space="PSUM") as ptpool,
            tc.tile_pool(name="pgp", bufs=3, space="PSUM") as pgpool,
            tc.tile_pool(name="pop", bufs=3, space="PSUM") as popool,
        ):
            emb_sb = cpool.tile([V, D], BF16, name="emb_sb")
            nc.sync.dma_start(emb_sb[:], emb.ap())
            identb = cpool.tile([128, 128], BF16, name="identb")
            nc.sync.dma_start(identb[:], ident.ap())
            w_bf = cpool.tile([128, 9 * D], BF16, name="w_bf")
            nc.sync.dma_start(
                w_bf[:].rearrange("p (l t e) -> p l t e", l=3, t=3), w_v
            )

            h0_by_b = {}

            def emit_prologue(st):
                """DMA + diagonal +I for (b, t); H0 block at each new b."""
                b, t = st["b"], st["t"]
                if t == 0:
                    oh_sb = ohpool.tile([V, NN], BF16, name="oh_sb")
                    nc.sync.dma_start(oh_sb[:], oh.ap()[b])
                    ph = popool.tile([128, NT * D], F32, name="ph", tag="po")
                    for ii in range(NT):
                        nc.tensor.matmul(
                            ph[:, ii * D : (ii + 1) * D],
                            lhsT=oh_sb[:, ii * 128 : (ii + 1) * 128],
                            rhs=emb_sb[:],
                            start=True,
                            stop=True,
                        )
                    h0 = h0pool.tile([128, NT * D], BF16, name="h0")
                    nc.scalar.copy(h0[:], ph[:])
                    h0_by_b[b] = h0
                e_bf = epool.tile([128, NT * NN], BF16, name="e_bf", tag="e")
                nc.sync.dma_start(
                    e_bf[:].rearrange("p (ii j) -> p ii j", ii=NT), e_v[b, t]
                )
                st["e_bf"] = e_bf

            def emit_reduce_chunk(st, ii):
                """Partial rowsum of E+I for node-block ii (small op, easy
                for the scheduler to slot between the relu/copy traffic)."""
                if ii == 0:
                    st["s1"] = degpool.tile([128, NT], F32, name="s1", tag="s1")
                nc.vector.tensor_reduce(
                    st["s1"][:, ii : ii + 1],
                    st["e_bf"][:].rearrange("p (ii j) -> p ii j", ii=NT)[
                        :, ii : ii + 1, :
                    ],
                    axis=mybir.AxisListType.X,
                    op=mybir.AluOpType.add,
                )

            def emit_degrees(st):
                """d2 = 1/rowsum, dd = sqrt(d2)."""
                d2 = degpool.tile([128, NT], F32, name="d2", tag="d2")
                nc.vector.reciprocal(d2[:], st["s1"][:])
                dd = degpool.tile([128, NT], F32, name="dd", tag="dd")
                nc.scalar.sqrt(dd[:], d2[:])
                st["d2"], st["dd"] = d2, dd

            def emit_diag(st):
                """+ I on diagonal blocks of e_bf (idle gpsimd), so Es^T
                includes the self-loops; the rowsum adds its +1 separately."""
                for ii in range(NT):
                    blk = slice(ii * NN + ii * 128, ii * NN + ii * 128 + 128)
                    nc.gpsimd.tensor_add(
                        st["e_bf"][:, blk], st["e_bf"][:, blk], identb[:]
                    )

            def emit_tgroup(st, jj):
                """4 PE transposes for column-block jj + its Es^T copy."""
                if jj == 0:
                    st["pt"] = ptpool.tile([128, NT * NN], BF16, name="pt", tag="pt")
                    st["est"] = estpool.tile(
                        [128, NT * NN], BF16, name="est", tag="est"
                    )
                pt, est, e_bf = st["pt"], st["est"], st["e_bf"]
                for ii in range(NT):
                    off = jj * NN + ii * 128
                    nc.tensor.transpose(
                        pt[:, off : off + 128],
                        e_bf[:, ii * NN + jj * 128 : ii * NN + jj * 128 + 128],
                        identb[:],
                    )
                # one copy per PAIR of column blocks: fewer ops + sem waits
                if jj == 1:
                    nc.vector.tensor_copy(est[:, : 2 * NN], pt[:, : 2 * NN])
                elif jj == 3:
                    nc.scalar.copy(est[:, 2 * NN :], pt[:, 2 * NN :])

            def emit_z0(st):
                """Z_0 = dd * H0 (one fused DVE broadcast multiply)."""
                z0 = zpool.tile([128, NT * D], BF16, name="z0", tag="z")
                nc.vector.tensor_tensor(
                    z0[:].rearrange("p (ii e) -> p ii e", ii=NT),
                    h0_by_b[st["b"]][:].rearrange("p (ii e) -> p ii e", ii=NT),
                    st["dd"][:].to_broadcast([128, NT, D]),
                    op=mybir.AluOpType.mult,
                )
                st["h"] = z0

            def emit_big(st, l):
                """G'T = (Es Z_l)^T: 4 accumulating matmuls into one bank."""
                pgt = pgpool.tile([128, NN], F32, name="pgt", tag="pg")
                h, est = st["h"], st["est"]
                for jj in range(NT):
                    nc.tensor.matmul(
                        pgt[:],
                        lhsT=h[:, jj * D : (jj + 1) * D],
                        rhs=est[:, jj * NN : (jj + 1) * NN],
                        start=(jj == 0),
                        stop=(jj == NT - 1),
                    )
                st["pgt"] = pgt

            def emit_gt(st, l):
                gt = gtpool.tile([128, NN], BF16, name="gt", tag="gt")
                nc.scalar.copy(gt[:], st["pgt"][:])
                st["gt"] = gt

            def emit_wmm(st, l):
                po = popool.tile([128, NT * D], F32, name="po", tag="po")
                gt = st["gt"]
                wsl = w_bf[:, (l * T + st["t"]) * D : (l * T + st["t"] + 1) * D]
                for ii in range(NT):
                    nc.tensor.matmul(
                        po[:, ii * D : (ii + 1) * D],
                        lhsT=gt[:, ii * 128 : (ii + 1) * 128],
                        rhs=wsl,
                        start=True,
                        stop=True,
                    )
                st["po"] = po

            def emit_relu(st, l):
                """Z_{l+1} = relu(po)*d2 (dd on the last layer), then out."""
                last = l == 2
                hn = zpool.tile(
                    [128, NT * D], F32 if last else BF16, name="hn", tag="z"
                )
                sc = st["dd"] if last else st["d2"]
                nc.vector.scalar_tensor_tensor(
                    hn[:].rearrange("p (ii e) -> p ii e", ii=NT),
                    st["po"][:].rearrange("p (ii e) -> p ii e", ii=NT),
                    0.0,
                    sc[:].to_broadcast([128, NT, D]),
                    op0=mybir.AluOpType.max,
                    op1=mybir.AluOpType.mult,
                )
                st["h"] = hn
                if last:
                    nc.scalar.dma_start(
                        out_v[st["b"], st["t"]],
                        hn[:].rearrange("p (ii e) -> p ii e", ii=NT),
                    )

            # 2-deep software pipeline: iteration k runs stage S0 (loads /
            # Es^T DMA-transpose / degrees / Z0) for bt_k, stage S1 (layer 0
            # + layer-1 matmul) for bt_{k-1}, and stage S2 (layer 1 epilogue
            # + layer 2 + store) for bt_{k-2}. Two independent layer chains
            # keep the PE busy across every cross-engine handoff, which also
            # keeps the HAM clock-gate warm.
            bts = [(b, t) for b in range(BC) for t in range(T)]
            sts = [{"b": b, "t": t} for b, t in bts]
            for k in range(len(bts) + 2):
                st = sts[k] if k < len(bts) else None
                A = sts[k - 1] if 1 <= k <= len(bts) else None
                B = sts[k - 2] if 2 <= k <= len(bts) + 1 else None
                if st:
                    emit_prologue(st)
                    emit_diag(st)
                if B:
                    emit_gt(B, 1)
                if st:
                    emit_tgroup(st, 0)
                    emit_reduce_chunk(st, 0)
                if A:
                    emit_big(A, 0)
                if B:
                    emit_wmm(B, 1)
                if st:
                    emit_tgroup(st, 1)
                    emit_reduce_chunk(st, 1)
                if A:
                    emit_gt(A, 0)
                if B:
                    emit_relu(B, 1)
                    emit_big(B, 2)
                if st:
                    emit_tgroup(st, 2)
                    emit_reduce_chunk(st, 2)
                if A:
                    emit_wmm(A, 0)
                if B:
                    emit_gt(B, 2)
                if A:
                    emit_relu(A, 0)
                if st:
                    emit_tgroup(st, 3)
                    emit_reduce_chunk(st, 3)
                if B:
                    emit_wmm(B, 2)
                if A:
                    emit_big(A, 1)
                if B:
                    emit_relu(B, 2)
                if st:
                    emit_degrees(st)
                    emit_z0(st)

    nc.compile()
    return nc


def _get_module(mm_mode: str) -> bass.Bass:
    if mm_mode not in _module_cache:
        if mm_mode == "bf16":
            _module_cache[mm_mode] = _build_module_v2()
        else:
            _module_cache[mm_mode] = _build_module(mm_mode)
    return _module_cache[mm_mode]


last_results = None


def kernel(**inputs) -> np.ndarray:
    N = np.asarray(inputs["N"])
    E = np.asarray(inputs["E"], dtype=np.float32)
    embed = np.ascontiguousarray(np.asarray(inputs["embed"], dtype=np.float32))
    W = np.ascontiguousarray(
        np.stack(
            [
                np.asarray(inputs["W1"], dtype=np.float32),
                np.asarray(inputs["W2"], dtype=np.float32),
                np.asarray(inputs["W3"], dtype=np.float32),
            ]
        )
    )  # [3, T, D, D]
    oh = (N[:, None, :] == np.arange(V)[None, :, None]).astype(np.float32)
    ident = np.eye(128, dtype=np.float32)

    if MM_MODE == "bf16":
        import ml_dtypes

        bf = ml_dtypes.bfloat16
        E = E.astype(bf)
        embed = embed.astype(bf)
        W = W.astype(bf)
        oh = oh.astype(bf)
        ident = ident.astype(bf)

    nc = _get_module(MM_MODE)
    in_maps = []
    for c in range(NCORES):
        sl = slice(c * BC, (c + 1) * BC)
        in_maps.append(
            {
                "e": np.ascontiguousarray(E[sl]),
                "oh": np.ascontiguousarray(oh[sl]),
                "emb": embed,
                "w": W,
                "ident": ident,
            }
        )

    trace = os.environ.get("KERNEL_TRACE", "") == "1"
    res = run_bass_kernel_spmd(
        nc,
        in_maps,
        core_ids=list(range(NCORES)),
        trace=trace,
    )
    global last_results
    last_results = res
    return np.concatenate([r["out"] for r in res.results], axis=0)
